# revision 46
# baseline (speedup 1.0000x reference)
"""Trainium2 Bass kernel for nn_Encoder_Flows (3-layer dense GCN message passing).

Math per graph (reference):
    A = flows [N, N];  deg[c] = sum_r A[r, c];  dinv = rsqrt(deg)
    L(x, W, b) = dinv * (A^T @ (dinv * (x @ W))) + b
    out = L(L(L(A, W1, b1), W2, b2), W3, b3)          # [N, 128]

Algebra: with M = diag(dinv) A^T diag(dinv), node-dim M commutes with the
feature-dim weights, so (bias-free) out = M^3 (A W1) (W2 W3).  Key trick of
this version: the degree normalization is folded into the shipped matrix on
the host:  Ahat = D A D  =>  M = Ahat^T exactly.  Every M-apply is then a
plain fp8 DoubleRow matmul chain  t_k = Ahat^T w_{k-1}  with NO per-step
dinv scaling on device (the vector engine only does psum->sbuf casts and the
eps residuals).  The U phase u = A W1 runs off Ahat^T strips with
W1g = D^{-1} W1 folded on the host: (D^{-1}W1)^T Ahat^T = u^T D, undone by a
per-node act scale when u is quantized.

fp8 quantization corrections (node-mean noise is amplified ~sqrt(N) by the
adjacency's Perron mode):
  - W1g is dither-quantized per column (error col-sums ~0).
  - w1/w2's quantization residual col-means mu_k are measured on device and
    applied as exact rank-1 psum accumulations  m1 (x) mu_k  into the next
    phase, where m1 = colsum(Ahat_q) is shipped from the host (a K=1 matmul
    appended to the accumulation group -- no vector work).
Scales: Ahat*2^16, W1g*2^6, w*2^7(t units), mu*2^-11, W23*2^2, out=psum*2^-15.

Performance design (measured baseline: PE never left the 1.2GHz mid p-state;
the 2.4GHz p-state needs >3us of gapless tensor-queue execution):
  - data: 2 graphs/core, Ahat shipped in both layouts (natural row-packed +
    transposed strips), 1MB c-chunk DMAs so each phase is chunk-paced;
    output in bf16.  ~17.9MB on a ~400GB/s DMA bus dominates the schedule.
  - the tensor queue is padded with junk DoubleRow "filler" matmuls wherever
    it would otherwise idle (DMA-paced stretches, post-processing trails) so
    the PE holds the full 2.4GHz clock; at full clock a 512-col DR matmul
    retires in ~107ns.
  - posts per chunk: vector casts psum->bf16, PE transposes 128x128 blocks,
    scalar quantizes to fp8 (act, with the per-node scale vector for U),
    vector computes eps = x - w in one scalar_tensor_tensor op.
"""

import sys
from contextlib import ExitStack

import numpy as np

for _p in ("/opt/trn_rl_repo", "/opt/pypackages"):
    if _p not in sys.path:
        sys.path.append(_p)

import ml_dtypes

B, N, P = 16, 2048, 128
NB = N // P          # 16 node 128-blocks
KB = N // (2 * P)    # 8 DoubleRow pair-blocks (256 rows each)
NCORES = 8
GPC = B // NCORES    # graphs per core
D = 128              # feature width carried through the fused chain
CH = 512             # psum chunk (one bank of fp32)
NCH = N // CH

_COMPILED = {}

# filler counts (each ~107ns of junk DR matmul at full clock), tuned to the
# DMA pacing of the schedule
FILL = dict(head=25, u0=4, u1=4, ut=4, t1=4)


def _build():
    import concourse.mybir as mybir
    import concourse.tile as tile
    from concourse import bacc

    f32 = mybir.dt.float32
    bf16 = mybir.dt.bfloat16
    fp8 = mybir.dt.float8e4
    DR = mybir.MatmulPerfMode.DoubleRow
    MUL = mybir.AluOpType.mult
    SUB = mybir.AluOpType.subtract
    COPY = mybir.ActivationFunctionType.Copy

    nc = bacc.Bacc("TRN2", target_bir_lowering=False)
    AH_d = nc.declare_dram_parameter("AH", [GPC, NCH, P, KB, 2, CH], fp8,
                                     isOutput=False)
    AT_d = nc.declare_dram_parameter("AT", [GPC, NCH, P, KB, 2, CH], fp8,
                                     isOutput=False)
    W1G_d = nc.declare_dram_parameter("W1G", [GPC, KB, P, 2, D], fp8,
                                      isOutput=False)
    W23_d = nc.declare_dram_parameter("W23", [P, D], bf16, isOutput=False)
    M1R_d = nc.declare_dram_parameter("M1R", [GPC, 1, N], bf16, isOutput=False)
    SC_d = nc.declare_dram_parameter("SC", [GPC, P, NB], f32, isOutput=False)
    IOB_d = nc.declare_dram_parameter("IOB", [P, P], bf16, isOutput=False)
    out_d = nc.declare_dram_parameter("out", [GPC, D, N], bf16, isOutput=True)

    with tile.TileContext(nc) as tc, ExitStack() as ctx:
        wpool = ctx.enter_context(tc.tile_pool(name="wpool", bufs=1))
        ahp = ctx.enter_context(tc.tile_pool(name="ahp", bufs=1))
        atp = ctx.enter_context(tc.tile_pool(name="atp", bufs=1))
        cpool = ctx.enter_context(tc.tile_pool(name="cpool", bufs=2))
        wqp = ctx.enter_context(tc.tile_pool(name="wqp", bufs=1))
        mpool = ctx.enter_context(tc.tile_pool(name="mpool", bufs=1))
        ogp = ctx.enter_context(tc.tile_pool(name="ogp", bufs=2))
        ps = ctx.enter_context(tc.tile_pool(name="ps", bufs=1, space="PSUM"))

        # ---- tiles ----
        W23 = wpool.tile([P, D], bf16)
        iob = wpool.tile([P, P], bf16)
        wg, sc, m1r = {}, {}, {}
        for g in range(GPC):
            wg[g] = wpool.tile([P, KB, 2, D], fp8, tag=f"wg{g}", name=f"wg{g}")
            sc[g] = wpool.tile([P, NB], f32, tag=f"sc{g}", name=f"sc{g}")
            m1r[g] = wpool.tile([1, N], bf16, tag=f"m1{g}", name=f"m1{g}")
        ones8 = wpool.tile([P, 2, 1], fp8)
        nc.vector.memset(ones8[:], 1.0)
        jW = wpool.tile([P, 2, P], fp8, tag="jW", name="jW")
        nc.vector.memset(jW[:], 0.0)
        jR = wpool.tile([P, 2, CH], fp8, tag="jR", name="jR")
        nc.vector.memset(jR[:], 0.0)

        AT, AH = {}, {}
        for g in range(GPC):
            AT[g] = [atp.tile([P, KB, 2, CH], fp8, tag=f"at{g}{r}",
                              name=f"at{g}{r}") for r in range(NCH)]
            AH[g] = [ahp.tile([P, KB, 2, CH], fp8, tag=f"ah{g}{c}",
                              name=f"ah{g}{c}") for c in range(NCH)]

        # ---- DMA issue order: small tensors, then both graphs' strips
        # (U phases run first), then the naturals.
        for g in range(GPC):
            nc.sync.dma_start(wg[g][:],
                              W1G_d.ap()[g].rearrange("kb p i d -> p kb i d"))
            nc.sync.dma_start(sc[g][:], SC_d.ap()[g])
            nc.sync.dma_start(m1r[g][:], M1R_d.ap()[g])
        nc.sync.dma_start(W23[:], W23_d.ap())
        nc.sync.dma_start(iob[:], IOB_d.ap())
        # chunks stream in kb-halves so each chunk's first 4 matmuls can
        # start when half the data has landed (subtile deps)
        H = KB // 2
        for g in range(GPC):
            for r in range(NCH):
                nc.sync.dma_start(AT[g][r][:, :H], AT_d.ap()[g][r][:, :H])
                nc.sync.dma_start(AT[g][r][:, H:], AT_d.ap()[g][r][:, H:])
        for g in range(GPC):
            for c in range(NCH):
                nc.sync.dma_start(AH[g][c][:, :H], AH_d.ap()[g][c][:, :H])
                nc.sync.dma_start(AH[g][c][:, H:], AH_d.ap()[g][c][:, H:])

        out_ap = out_d.ap()

        ptag = {0: [f"a{i}" for i in range(NCH)], 1: [f"b{i}" for i in range(NCH)]}
        fillt = {}

        def fill(n, g_idle):
            """n junk DR matmuls (~107ns each at full clock) to keep the PE
            queue gapless; writes a scratch bank of the idle graph."""
            t = fillt.get(g_idle)
            if t is None:
                t = ps.tile([P, CH], f32, tag=ptag[g_idle][3],
                            name=f"fill{g_idle}")
                fillt[g_idle] = t
            for _ in range(n):
                nc.tensor.matmul(t[:], jW[:], jR[:], start=True, stop=True,
                                 perf_mode=DR, skip_group_check=True)

        w8 = {}       # current fp8 lhsT per graph
        xsum = {}     # [P, NCH] f32 colsum(x) accums per graph
        murow = {}    # mu row [1, D] bf16 per (g, step)

        def u_mms(g, rch):
            ups = ps.tile([P, CH], f32, tag=ptag[g][rch], name=f"ups{g}{rch}")
            for cb in range(KB):
                nc.tensor.matmul(ups[:], wg[g][:, cb], AT[g][rch][:, cb],
                                 start=(cb == 0), stop=(cb == KB - 1),
                                 perf_mode=DR)
            return ups

        def _transposes(g, ch, tf, name):
            tq = ps.tile([P, 4, P], bf16, tag=ptag[g][ch], name=f"tqp{name}")
            for j in range(4):
                nc.tensor.transpose(tq[:, j], tf[:, j * P:(j + 1) * P],
                                    iob[:])
            return tq

        def u_post(g, rch, ups, w):
            sl = slice(rch * 4, rch * 4 + 4)
            tf = cpool.tile([P, CH], bf16, tag=f"tf{g}{rch % 2}",
                            name=f"utf{g}{rch}")
            nc.scalar.activation(tf[:], ups[:], COPY)
            tq = _transposes(g, rch, tf, f"u{g}{rch}")
            nc.vector.tensor_tensor(
                w[:, sl], tq[:],
                sc[g][:, sl, None].to_broadcast([P, 4, D]), MUL)

        def t_mms(g, step, ch):
            tps = ps.tile([P, CH], f32, tag=ptag[g][ch], name=f"t{step}{g}{ch}")
            for kb in range(KB):
                nc.tensor.matmul(tps[:], w8[g][:, 2 * kb:2 * kb + 2],
                                 AH[g][ch][:, kb],
                                 start=(kb == 0), stop=(kb == KB - 1),
                                 perf_mode=DR)
                if step > 1 and kb == KB - 2:
                    # rank-1 mu correction, accumulated inside the group
                    nc.tensor.matmul(tps[:], murow[(g, step - 1)],
                                     m1r[g][:, ch * CH:(ch + 1) * CH],
                                     start=False, stop=False,
                                     skip_group_check=True)
            return tps

        def t_post(g, step, ch, tps, wn, xs):
            sl = slice(ch * 4, ch * 4 + 4)
            # scalar casts psum -> bf16 in w units (x2^-16); for steps 1-2
            # accum_out captures colsum(x) over this chunk's nodes for mu
            tf = cpool.tile([P, CH], bf16, tag=f"tf{g}{ch % 2}",
                            name=f"tf{g}{step}{ch}")
            if step == 3:
                nc.scalar.activation(tf[:], tps[:], COPY, scale=2.0 ** -16)
                # feature-major epilogue: out^T-chunk = W23^T @ tf in one
                # 512-col matmul; the host transposes the final output.
                epo = ps.tile([P, CH], f32, tag=ptag[g][ch], name=f"epo{g}{ch}")
                nc.tensor.matmul(epo[:], W23[:], tf[:], start=True, stop=True)
                og = ogp.tile([P, CH], bf16, tag=f"og{g}", name=f"og{g}{ch}")
                nc.vector.tensor_scalar_mul(og[:], epo[:], 2.0 ** -15)
                nc.sync.dma_start(out_ap[g][:, ch * CH:(ch + 1) * CH], og[:])
                return
            nc.scalar.activation(tf[:], tps[:], COPY, scale=2.0 ** -16,
                                 accum_out=xs[:, ch, None])
            tq = _transposes(g, ch, tf, f"t{g}{step}{ch}")
            nc.vector.tensor_copy(wn[:, sl], tq[:])

        def phase_u(g, g_idle, nfill, depth=2):
            w = wqp.tile([P, NB, D], fp8, tag=f"w{g}0", name=f"w0{g}")
            pend = []
            for rch in range(NCH):
                pend.append((rch, u_mms(g, rch)))
                if len(pend) > depth:
                    r_, u_ = pend.pop(0)
                    u_post(g, r_, u_, w)
                if rch < NCH - 1:
                    fill(nfill, g_idle)
            for r_, u_ in pend:
                u_post(g, r_, u_, w)
            w8[g] = w

        def phase_t(g, g_idle, step, nfill, depth=1):
            wn = xs = None
            if step < 3:
                wn = wqp.tile([P, NB, D], fp8, tag=f"w{g}{step % 2}",
                              name=f"w{step}{g}")
                xs = mpool.tile([P, NCH], f32, tag=f"xs{g}",
                                name=f"xs{step}{g}")
            pend = []
            for ch in range(NCH):
                pend.append((ch, t_mms(g, step, ch)))
                if len(pend) > depth:
                    c_, t_ = pend.pop(0)
                    t_post(g, step, c_, t_, wn, xs)
                if nfill and ch < NCH - 1:
                    fill(nfill, g_idle)
            for c_, t_ in pend:
                t_post(g, step, c_, t_, wn, xs)
            if step < 3:
                w8[g], xsum[g] = wn, xs

        def phase_mu(g, step):
            """mu = 2^-11 * (colsum(x) - colsum(w)) as a [1, D] bf16 row.
            colsum(x) came free from the cast accum_out; colsum(w) is 8 DR
            matmuls of the just-quantized w8 against ones."""
            muT = ps.tile([P, 1], f32, tag=ptag[g][0], name=f"muT{g}{step}")
            for m in range(KB):
                nc.tensor.matmul(muT[:], w8[g][:, 2 * m:2 * m + 2], ones8[:],
                                 start=(m == 0), stop=(m == KB - 1),
                                 perf_mode=DR)
            xr = mpool.tile([P, 1], f32, tag=f"xr{g}", name=f"xr{g}{step}")
            nc.vector.tensor_reduce(xr[:], xsum[g][:], mybir.AxisListType.X,
                                    mybir.AluOpType.add)
            muTs = mpool.tile([P, 1], bf16, tag=f"muTs{g}", name=f"muTs{g}{step}")
            nc.vector.tensor_tensor(muTs[:], xr[:], muT[:], SUB)
            rowp = ps.tile([1, P], bf16, tag=ptag[g][0], name=f"murp{g}{step}")
            nc.tensor.transpose(rowp[:], muTs[:], iob[:])
            row = mpool.tile([1, P], bf16, tag=f"mur{g}{step}",
                             name=f"mur{g}{step}")
            nc.vector.tensor_copy(row[:], rowp[:])
            murow[(g, step)] = row

        # ---- schedule ----
        # stream order is AT0, AT1, AH0, AH1: both U phases run up front
        # against the strip stream; T1(0) is paced by AH0; T2(0) interleaves
        # chunk-by-chunk with the AH1-paced T1(1); the rest runs free.
        fill(FILL["head"], 1)
        phase_u(0, 1, FILL["u0"])
        phase_u(1, 0, FILL["u1"])
        fill(FILL["ut"], 1)
        phase_t(0, 1, 1, FILL["t1"], depth=2)
        phase_mu(0, 1)
        w2t = wqp.tile([P, NB, D], fp8, tag="w00", name="w20")
        xs2 = mpool.tile([P, NCH], f32, tag="xs0", name="xs20")
        wt1 = wqp.tile([P, NB, D], fp8, tag="w11", name="w11")
        xst = mpool.tile([P, NCH], f32, tag="xs1", name="xs11")
        pend = []
        for ch in range(NCH):
            pend.append((0, 2, ch, t_mms(0, 2, ch), w2t, xs2))
            pend.append((1, 1, ch, t_mms(1, 1, ch), wt1, xst))
            while len(pend) > 2:
                g_, s_, c_, tps_, w_, x_ = pend.pop(0)
                t_post(g_, s_, c_, tps_, w_, x_)
        for g_, s_, c_, tps_, w_, x_ in pend:
            t_post(g_, s_, c_, tps_, w_, x_)
        w8[0], xsum[0] = w2t, xs2
        phase_mu(0, 2)
        w8[1], xsum[1] = wt1, xst
        phase_mu(1, 1)
        # T3(0) interleaves chunk-by-chunk with T2(1)
        w2u = wqp.tile([P, NB, D], fp8, tag="w10", name="w21")
        xs2u = mpool.tile([P, NCH], f32, tag="xs1", name="xs21")
        pend = []
        for ch in range(NCH):
            pend.append((0, 3, ch, t_mms(0, 3, ch), None, None))
            pend.append((1, 2, ch, t_mms(1, 2, ch), w2u, xs2u))
            while len(pend) > 2:
                g_, s_, c_, tps_, w_, x_ = pend.pop(0)
                t_post(g_, s_, c_, tps_, w_, x_)
        for g_, s_, c_, tps_, w_, x_ in pend:
            t_post(g_, s_, c_, tps_, w_, x_)
        w8[1], xsum[1] = w2u, xs2u
        phase_mu(1, 2)
        phase_t(1, 0, 3, 0)

    nc.compile()
    return nc


def _get_nc():
    if "nc" not in _COMPILED:
        _COMPILED["nc"] = _build()
    return _COMPILED["nc"]


FP8 = ml_dtypes.float8_e4m3
BF16 = ml_dtypes.bfloat16


def _q8(x):
    return np.clip(x, -240.0, 240.0).astype(FP8)


def _dither_q8(xs):
    """Per-column fp8 quantization with near-zero column error means."""
    q = _q8(xs)
    qf = q.astype(np.float32)
    r = xs - qf
    m = r.sum(0)
    s = np.where(m >= 0, 1.0, -1.0).astype(np.float32)
    u = q.view(np.uint8)
    mag = (u & 0x7F).astype(np.int16)
    neg = (u & 0x80) != 0
    dirpos = np.broadcast_to(s > 0, xs.shape)
    away = (~neg) == dirpos
    nmag = np.where(mag == 0, 1, np.where(away, mag + 1, mag - 1))
    nsign = np.where(mag == 0, ~dirpos, neg)
    nb = ((nmag.astype(np.uint8) & 0x7F) | (nsign.astype(np.uint8) << 7))
    nxt = nb.view(FP8).astype(np.float32)
    ok = np.isfinite(nxt) & (np.abs(nxt) <= 240.0) & (nmag <= 0x7E)
    step = np.where(ok, nxt - qf, 0.0)
    key = np.where(ok, r * s[None, :], -np.inf)
    order = np.argsort(-key, axis=0)
    step_sorted = np.take_along_axis(step, order, axis=0)
    cum = np.cumsum(step_sorted, axis=0)
    err = np.abs(m[None, :] - cum)
    k = np.argmin(np.vstack([np.abs(m)[None, :], err]), axis=0)
    out = qf.copy()
    for d in range(xs.shape[1]):
        if k[d] > 0:
            idx = order[:k[d], d]
            out[idx, d] = nxt[idx, d]
    return out.astype(FP8)


def _pack(a):
    """[B, N(rows), N(cols)] -> [B, NCH, P, KB, 2, CH]: rows r = 256kb+128i+p
    packed DoubleRow, cols c = 512ch + cc chunked."""
    x = a.reshape(B, KB, 2, P, NCH, CH)
    return np.ascontiguousarray(x.transpose(0, 4, 3, 1, 2, 5))


def kernel(flows, W1, b1, W2, b2, W3, b3, _trace=False):
    from concourse.bass_utils import run_bass_kernel_spmd

    flows = np.asarray(flows, dtype=np.float32)
    W1 = np.asarray(W1, dtype=np.float32)
    W2 = np.asarray(W2, dtype=np.float32)
    W3 = np.asarray(W3, dtype=np.float32)
    b1 = np.asarray(b1, dtype=np.float32)
    b2 = np.asarray(b2, dtype=np.float32)
    b3 = np.asarray(b3, dtype=np.float32)

    nc = _get_nc()

    deg = flows.sum(axis=1)                          # [B, N] column sums
    dinv = (1.0 / np.sqrt(deg)).astype(np.float32)

    # Ahat = D A D, quantized at 2^16; natural + transposed packings
    Aq8 = np.empty((B, N, N), dtype=FP8)
    M1R = np.empty((B, 1, N), dtype=BF16)
    for g in range(B):
        ah = (flows[g] * (dinv[g][:, None] * 2.0 ** 16)) * dinv[g][None, :]
        Aq8[g] = _q8(ah)
        M1R[g, 0] = (Aq8[g].astype(np.float32).sum(axis=0)
                     * 2.0 ** -11).astype(BF16)
    AHp = _pack(Aq8)
    ATp = _pack(np.ascontiguousarray(Aq8.transpose(0, 2, 1)))

    # W1g = 2^6 D^{-1} W1 per graph, dither-quantized per column
    W1g = (np.sqrt(deg)[:, :, None] * W1[None, :, :] * 2.0 ** 6).astype(np.float32)
    W1q = _dither_q8(W1g.transpose(1, 0, 2).reshape(N, B * D))
    W1q = W1q.reshape(N, B, D).transpose(1, 0, 2)    # [B, N, D] fp8
    W1G = np.ascontiguousarray(
        W1q.reshape(B, KB, 2, P, D).transpose(0, 1, 3, 2, 4))

    W23 = ((W2 @ W3) * 2.0 ** 9).astype(BF16)
    SC = (np.sqrt(deg) * 2.0 ** -16).astype(np.float32)       # [B, N]
    SC = np.ascontiguousarray(SC.reshape(B, NB, P).transpose(0, 2, 1))

    in_maps = []
    for c in range(NCORES):
        sl = slice(c * GPC, (c + 1) * GPC)
        in_maps.append({
            "AH": AHp[sl], "AT": ATp[sl],
            "W1G": W1G[sl], "W23": W23,
            "M1R": M1R[sl], "SC": SC[sl],
            "IOB": np.eye(P, dtype=BF16),
        })

    res = run_bass_kernel_spmd(nc, in_maps, core_ids=list(range(NCORES)),
                               trace=_trace)
    out = np.concatenate([res.results[c]["out"] for c in range(NCORES)], axis=0)
    out = np.ascontiguousarray(out.astype(np.float32).transpose(0, 2, 1))

    if np.any(b1) or np.any(b2) or np.any(b3):
        dv = np.where(deg > 0, 1.0 / np.sqrt(deg), 0.0).astype(np.float32)
        m1 = dv * np.einsum('brc,br->bc', flows, dv)
        m2 = dv * np.einsum('brc,br->bc', flows, dv * m1)
        out += m2[..., None] * (b1 @ W2 @ W3)[None, None, :]
        out += m1[..., None] * (b2 @ W3)[None, None, :]
        out += b3[None, None, :]

    if _trace:
        return out, res
    return out


# revision 50
# speedup vs baseline: 1.0409x; 1.0409x over previous
"""Trainium2 Bass kernel for nn_Encoder_Flows (3-layer dense GCN message passing).

Math per graph (reference):
    A = flows [N, N];  deg[c] = sum_r A[r, c];  dinv = rsqrt(deg)
    L(x, W, b) = dinv * (A^T @ (dinv * (x @ W))) + b
    out = L(L(L(A, W1, b1), W2, b2), W3, b3)          # [N, 128]

Algebra: with M = diag(dinv) A^T diag(dinv), node-dim M commutes with the
feature-dim weights, so (bias-free) out = M^3 (A W1) (W2 W3).  Key trick of
this version: the degree normalization is folded into the shipped matrix on
the host:  Ahat = D A D  =>  M = Ahat^T exactly.  Every M-apply is then a
plain fp8 DoubleRow matmul chain  t_k = Ahat^T w_{k-1}  with NO per-step
dinv scaling on device (the vector engine only does psum->sbuf casts and the
eps residuals).  The U phase u = A W1 runs off Ahat^T strips with
W1g = D^{-1} W1 folded on the host: (D^{-1}W1)^T Ahat^T = u^T D, undone by a
per-node act scale when u is quantized.

fp8 quantization corrections (node-mean noise is amplified ~sqrt(N) by the
adjacency's Perron mode):
  - W1g is dither-quantized per column (error col-sums ~0).
  - w1/w2's quantization residual col-means mu_k are measured on device and
    applied as exact rank-1 psum accumulations  m1 (x) mu_k  into the next
    phase, where m1 = colsum(Ahat_q) is shipped from the host (a K=1 matmul
    appended to the accumulation group -- no vector work).
Scales: Ahat*2^16, W1g*2^6, w*2^7(t units), mu*2^-11, W23*2^2, out=psum*2^-15.

Performance design (measured baseline: PE never left the 1.2GHz mid p-state;
the 2.4GHz p-state needs >3us of gapless tensor-queue execution):
  - data: 2 graphs/core, Ahat shipped in both layouts (natural row-packed +
    transposed strips), 1MB c-chunk DMAs so each phase is chunk-paced;
    output in bf16.  ~17.9MB on a ~400GB/s DMA bus dominates the schedule.
  - the tensor queue is padded with junk DoubleRow "filler" matmuls wherever
    it would otherwise idle (DMA-paced stretches, post-processing trails) so
    the PE holds the full 2.4GHz clock; at full clock a 512-col DR matmul
    retires in ~107ns.
  - posts per chunk: vector casts psum->bf16, PE transposes 128x128 blocks,
    scalar quantizes to fp8 (act, with the per-node scale vector for U),
    vector computes eps = x - w in one scalar_tensor_tensor op.
"""

import sys
from contextlib import ExitStack

import numpy as np

for _p in ("/opt/trn_rl_repo", "/opt/pypackages"):
    if _p not in sys.path:
        sys.path.append(_p)

import ml_dtypes

B, N, P = 16, 2048, 128
NB = N // P          # 16 node 128-blocks
KB = N // (2 * P)    # 8 DoubleRow pair-blocks (256 rows each)
NCORES = 8
GPC = B // NCORES    # graphs per core
D = 128              # feature width carried through the fused chain
CH = 512             # psum chunk (one bank of fp32)
NCH = N // CH

_COMPILED = {}

# filler counts (each ~107ns of junk DR matmul at full clock), tuned to the
# DMA pacing of the schedule
FILL = dict(head=25, u0=4, u1=4, ut=4, t1=4)


def _build():
    import concourse.mybir as mybir
    import concourse.tile as tile
    from concourse import bacc

    f32 = mybir.dt.float32
    bf16 = mybir.dt.bfloat16
    fp8 = mybir.dt.float8e4
    DR = mybir.MatmulPerfMode.DoubleRow
    MUL = mybir.AluOpType.mult
    SUB = mybir.AluOpType.subtract
    COPY = mybir.ActivationFunctionType.Copy

    nc = bacc.Bacc("TRN2", target_bir_lowering=False)
    AH_d = nc.declare_dram_parameter("AH", [GPC, NCH, P, KB, 2, CH], fp8,
                                     isOutput=False)
    AT_d = nc.declare_dram_parameter("AT", [GPC, NCH, P, KB, 2, CH], fp8,
                                     isOutput=False)
    W1G_d = nc.declare_dram_parameter("W1G", [GPC, KB, P, 2, D], fp8,
                                      isOutput=False)
    W23_d = nc.declare_dram_parameter("W23", [P, D], bf16, isOutput=False)
    M1R_d = nc.declare_dram_parameter("M1R", [GPC, 1, N], bf16, isOutput=False)
    SC_d = nc.declare_dram_parameter("SC", [GPC, P, NB], f32, isOutput=False)
    IOB_d = nc.declare_dram_parameter("IOB", [P, P], bf16, isOutput=False)
    out_d = nc.declare_dram_parameter("out", [GPC, D, N], bf16, isOutput=True)

    with tile.TileContext(nc) as tc, ExitStack() as ctx:
        wpool = ctx.enter_context(tc.tile_pool(name="wpool", bufs=1))
        ahp = ctx.enter_context(tc.tile_pool(name="ahp", bufs=1))
        atp = ctx.enter_context(tc.tile_pool(name="atp", bufs=1))
        cpool = ctx.enter_context(tc.tile_pool(name="cpool", bufs=2))
        wqp = ctx.enter_context(tc.tile_pool(name="wqp", bufs=1))
        mpool = ctx.enter_context(tc.tile_pool(name="mpool", bufs=1))
        ogp = ctx.enter_context(tc.tile_pool(name="ogp", bufs=2))
        ps = ctx.enter_context(tc.tile_pool(name="ps", bufs=1, space="PSUM"))

        # ---- tiles ----
        W23 = wpool.tile([P, D], bf16)
        iob = wpool.tile([P, P], bf16)
        wg, sc, m1r = {}, {}, {}
        for g in range(GPC):
            wg[g] = wpool.tile([P, KB, 2, D], fp8, tag=f"wg{g}", name=f"wg{g}")
            sc[g] = wpool.tile([P, NB], f32, tag=f"sc{g}", name=f"sc{g}")
            m1r[g] = wpool.tile([1, N], bf16, tag=f"m1{g}", name=f"m1{g}")
        ones8 = wpool.tile([P, 2, 1], fp8)
        nc.vector.memset(ones8[:], 1.0)
        jW = wpool.tile([P, 2, P], fp8, tag="jW", name="jW")
        nc.vector.memset(jW[:], 0.0)
        jR = wpool.tile([P, 2, CH], fp8, tag="jR", name="jR")
        nc.vector.memset(jR[:], 0.0)

        AT, AH = {}, {}
        for g in range(GPC):
            AT[g] = [atp.tile([P, KB, 2, CH], fp8, tag=f"at{g}{r}",
                              name=f"at{g}{r}") for r in range(NCH)]
            AH[g] = [ahp.tile([P, KB, 2, CH], fp8, tag=f"ah{g}{c}",
                              name=f"ah{g}{c}") for c in range(NCH)]

        # ---- DMA issue order: small tensors, then both graphs' strips
        # (U phases run first), then the naturals.
        for g in range(GPC):
            nc.sync.dma_start(wg[g][:],
                              W1G_d.ap()[g].rearrange("kb p i d -> p kb i d"))
            nc.sync.dma_start(sc[g][:], SC_d.ap()[g])
            nc.sync.dma_start(m1r[g][:], M1R_d.ap()[g])
        nc.sync.dma_start(W23[:], W23_d.ap())
        nc.sync.dma_start(iob[:], IOB_d.ap())
        # chunks stream in kb-halves so each chunk's first 4 matmuls can
        # start when half the data has landed (subtile deps)
        H = KB // 2
        for g in range(GPC):
            for r in range(NCH):
                nc.sync.dma_start(AT[g][r][:, :H], AT_d.ap()[g][r][:, :H])
                nc.sync.dma_start(AT[g][r][:, H:], AT_d.ap()[g][r][:, H:])
        for g in range(GPC):
            for c in range(NCH):
                nc.sync.dma_start(AH[g][c][:, :H], AH_d.ap()[g][c][:, :H])
                nc.sync.dma_start(AH[g][c][:, H:], AH_d.ap()[g][c][:, H:])

        out_ap = out_d.ap()

        ptag = {0: [f"a{i}" for i in range(NCH)], 1: [f"b{i}" for i in range(NCH)]}
        fillt = {}

        def fill(n, g_idle):
            """n junk DR matmuls (~107ns each at full clock) to keep the PE
            queue gapless; writes a scratch bank of the idle graph."""
            t = fillt.get(g_idle)
            if t is None:
                t = ps.tile([P, CH], f32, tag=ptag[g_idle][3],
                            name=f"fill{g_idle}")
                fillt[g_idle] = t
            for _ in range(n):
                nc.tensor.matmul(t[:], jW[:], jR[:], start=True, stop=True,
                                 perf_mode=DR, skip_group_check=True)

        w8 = {}       # current fp8 lhsT per graph
        xsum = {}     # [P, NCH] f32 colsum(x) accums per graph
        murow = {}    # mu row [1, D] bf16 per (g, step)

        def u_mms(g, rch):
            ups = ps.tile([P, CH], f32, tag=ptag[g][rch], name=f"ups{g}{rch}")
            for cb in range(KB):
                nc.tensor.matmul(ups[:], wg[g][:, cb], AT[g][rch][:, cb],
                                 start=(cb == 0), stop=(cb == KB - 1),
                                 perf_mode=DR)
            return ups

        def _transposes(g, ch, tf, name):
            tq = ps.tile([P, 4, P], bf16, tag=ptag[g][ch], name=f"tqp{name}")
            for j in range(4):
                nc.tensor.transpose(tq[:, j], tf[:, j * P:(j + 1) * P],
                                    iob[:])
            return tq

        def u_post(g, rch, ups, w):
            sl = slice(rch * 4, rch * 4 + 4)
            tf = cpool.tile([P, CH], bf16, tag=f"tf{g}{rch % 2}",
                            name=f"utf{g}{rch}")
            nc.scalar.activation(tf[:], ups[:], COPY)
            tq = _transposes(g, rch, tf, f"u{g}{rch}")
            nc.vector.tensor_tensor(
                w[:, sl], tq[:],
                sc[g][:, sl, None].to_broadcast([P, 4, D]), MUL)

        def t_mms(g, step, ch):
            tps = ps.tile([P, CH], f32, tag=ptag[g][ch], name=f"t{step}{g}{ch}")
            for kb in range(KB):
                nc.tensor.matmul(tps[:], w8[g][:, 2 * kb:2 * kb + 2],
                                 AH[g][ch][:, kb],
                                 start=(kb == 0), stop=(kb == KB - 1),
                                 perf_mode=DR)
                if step > 1 and kb == KB - 2:
                    # rank-1 mu correction, accumulated inside the group
                    nc.tensor.matmul(tps[:], murow[(g, step - 1)],
                                     m1r[g][:, ch * CH:(ch + 1) * CH],
                                     start=False, stop=False,
                                     skip_group_check=True)
            return tps

        def t_post(g, step, ch, tps, wn, xs):
            sl = slice(ch * 4, ch * 4 + 4)
            # scalar casts psum -> bf16 in w units (x2^-16); for steps 1-2
            # accum_out captures colsum(x) over this chunk's nodes for mu
            tf = cpool.tile([P, CH], bf16, tag=f"tf{g}{ch % 2}",
                            name=f"tf{g}{step}{ch}")
            if step == 3:
                nc.scalar.activation(tf[:], tps[:], COPY, scale=2.0 ** -16)
                # feature-major epilogue: out^T-chunk = W23^T @ tf in one
                # 512-col matmul; the host transposes the final output.
                epo = ps.tile([P, CH], f32, tag=ptag[g][ch], name=f"epo{g}{ch}")
                nc.tensor.matmul(epo[:], W23[:], tf[:], start=True, stop=True)
                og = ogp.tile([P, CH], bf16, tag=f"og{g}", name=f"og{g}{ch}")
                nc.vector.tensor_scalar_mul(og[:], epo[:], 2.0 ** -15)
                nc.sync.dma_start(out_ap[g][:, ch * CH:(ch + 1) * CH], og[:])
                return
            nc.scalar.activation(tf[:], tps[:], COPY, scale=2.0 ** -16,
                                 accum_out=xs[:, ch, None])
            tq = _transposes(g, ch, tf, f"t{g}{step}{ch}")
            nc.vector.tensor_copy(wn[:, sl], tq[:])

        def phase_u(g, g_idle, nfill, depth=1):
            w = wqp.tile([P, NB, D], fp8, tag=f"w{g}0", name=f"w0{g}")
            pend = []
            for rch in range(NCH):
                pend.append((rch, u_mms(g, rch)))
                if rch < NCH - 1:
                    fill(nfill, g_idle)
                if len(pend) > depth:
                    r_, u_ = pend.pop(0)
                    u_post(g, r_, u_, w)
            for r_, u_ in pend:
                u_post(g, r_, u_, w)
            w8[g] = w

        def phase_t(g, g_idle, step, nfill, depth=1):
            wn = xs = None
            if step < 3:
                wn = wqp.tile([P, NB, D], fp8, tag=f"w{g}{step % 2}",
                              name=f"w{step}{g}")
                xs = mpool.tile([P, NCH], f32, tag=f"xs{g}",
                                name=f"xs{step}{g}")
            pend = []
            for ch in range(NCH):
                pend.append((ch, t_mms(g, step, ch)))
                if nfill and ch < NCH - 1:
                    fill(nfill, g_idle)
                if len(pend) > depth:
                    c_, t_ = pend.pop(0)
                    t_post(g, step, c_, t_, wn, xs)
            for c_, t_ in pend:
                t_post(g, step, c_, t_, wn, xs)
            if step < 3:
                w8[g], xsum[g] = wn, xs

        def phase_mu(g, step):
            """mu = 2^-11 * (colsum(x) - colsum(w)) as a [1, D] bf16 row.
            colsum(x) came free from the cast accum_out; colsum(w) is 8 DR
            matmuls of the just-quantized w8 against ones."""
            muT = ps.tile([P, 1], f32, tag=ptag[g][0], name=f"muT{g}{step}")
            for m in range(KB):
                nc.tensor.matmul(muT[:], w8[g][:, 2 * m:2 * m + 2], ones8[:],
                                 start=(m == 0), stop=(m == KB - 1),
                                 perf_mode=DR)
            xr = mpool.tile([P, 1], f32, tag=f"xr{g}", name=f"xr{g}{step}")
            nc.vector.tensor_reduce(xr[:], xsum[g][:], mybir.AxisListType.X,
                                    mybir.AluOpType.add)
            muTs = mpool.tile([P, 1], bf16, tag=f"muTs{g}", name=f"muTs{g}{step}")
            nc.vector.tensor_tensor(muTs[:], xr[:], muT[:], SUB)
            rowp = ps.tile([1, P], bf16, tag=ptag[g][0], name=f"murp{g}{step}")
            nc.tensor.transpose(rowp[:], muTs[:], iob[:])
            row = mpool.tile([1, P], bf16, tag=f"mur{g}{step}",
                             name=f"mur{g}{step}")
            nc.vector.tensor_copy(row[:], rowp[:])
            murow[(g, step)] = row

        # ---- schedule ----
        # stream order is AT0, AT1, AH0, AH1: both U phases run up front
        # against the strip stream; T1(0) is paced by AH0; T2(0) interleaves
        # chunk-by-chunk with the AH1-paced T1(1); the rest runs free.
        fill(FILL["head"], 1)
        phase_u(0, 1, FILL["u0"])
        phase_u(1, 0, FILL["u1"])
        fill(FILL["ut"], 1)
        phase_t(0, 1, 1, FILL["t1"])
        phase_mu(0, 1)
        w2t = wqp.tile([P, NB, D], fp8, tag="w00", name="w20")
        xs2 = mpool.tile([P, NCH], f32, tag="xs0", name="xs20")
        wt1 = wqp.tile([P, NB, D], fp8, tag="w11", name="w11")
        xst = mpool.tile([P, NCH], f32, tag="xs1", name="xs11")
        pend = []
        for ch in range(NCH):
            pend.append((0, 2, ch, t_mms(0, 2, ch), w2t, xs2))
            pend.append((1, 1, ch, t_mms(1, 1, ch), wt1, xst))
            while len(pend) > 2:
                g_, s_, c_, tps_, w_, x_ = pend.pop(0)
                t_post(g_, s_, c_, tps_, w_, x_)
        for g_, s_, c_, tps_, w_, x_ in pend:
            t_post(g_, s_, c_, tps_, w_, x_)
        w8[0], xsum[0] = w2t, xs2
        phase_mu(0, 2)
        w8[1], xsum[1] = wt1, xst
        phase_mu(1, 1)
        # T3(0) interleaves chunk-by-chunk with T2(1)
        w2u = wqp.tile([P, NB, D], fp8, tag="w10", name="w21")
        xs2u = mpool.tile([P, NCH], f32, tag="xs1", name="xs21")
        pend = []
        for ch in range(NCH):
            pend.append((0, 3, ch, t_mms(0, 3, ch), None, None))
            pend.append((1, 2, ch, t_mms(1, 2, ch), w2u, xs2u))
            while len(pend) > 2:
                g_, s_, c_, tps_, w_, x_ = pend.pop(0)
                t_post(g_, s_, c_, tps_, w_, x_)
        for g_, s_, c_, tps_, w_, x_ in pend:
            t_post(g_, s_, c_, tps_, w_, x_)
        w8[1], xsum[1] = w2u, xs2u
        phase_mu(1, 2)
        phase_t(1, 0, 3, 0)

    nc.compile()
    return nc


def _get_nc():
    if "nc" not in _COMPILED:
        _COMPILED["nc"] = _build()
    return _COMPILED["nc"]


FP8 = ml_dtypes.float8_e4m3
BF16 = ml_dtypes.bfloat16


def _q8(x):
    return np.clip(x, -240.0, 240.0).astype(FP8)


def _dither_q8(xs):
    """Per-column fp8 quantization with near-zero column error means."""
    q = _q8(xs)
    qf = q.astype(np.float32)
    r = xs - qf
    m = r.sum(0)
    s = np.where(m >= 0, 1.0, -1.0).astype(np.float32)
    u = q.view(np.uint8)
    mag = (u & 0x7F).astype(np.int16)
    neg = (u & 0x80) != 0
    dirpos = np.broadcast_to(s > 0, xs.shape)
    away = (~neg) == dirpos
    nmag = np.where(mag == 0, 1, np.where(away, mag + 1, mag - 1))
    nsign = np.where(mag == 0, ~dirpos, neg)
    nb = ((nmag.astype(np.uint8) & 0x7F) | (nsign.astype(np.uint8) << 7))
    nxt = nb.view(FP8).astype(np.float32)
    ok = np.isfinite(nxt) & (np.abs(nxt) <= 240.0) & (nmag <= 0x7E)
    step = np.where(ok, nxt - qf, 0.0)
    key = np.where(ok, r * s[None, :], -np.inf)
    order = np.argsort(-key, axis=0)
    step_sorted = np.take_along_axis(step, order, axis=0)
    cum = np.cumsum(step_sorted, axis=0)
    err = np.abs(m[None, :] - cum)
    k = np.argmin(np.vstack([np.abs(m)[None, :], err]), axis=0)
    out = qf.copy()
    for d in range(xs.shape[1]):
        if k[d] > 0:
            idx = order[:k[d], d]
            out[idx, d] = nxt[idx, d]
    return out.astype(FP8)


def _pack(a):
    """[B, N(rows), N(cols)] -> [B, NCH, P, KB, 2, CH]: rows r = 256kb+128i+p
    packed DoubleRow, cols c = 512ch + cc chunked."""
    x = a.reshape(B, KB, 2, P, NCH, CH)
    return np.ascontiguousarray(x.transpose(0, 4, 3, 1, 2, 5))


def kernel(flows, W1, b1, W2, b2, W3, b3, _trace=False):
    from concourse.bass_utils import run_bass_kernel_spmd

    flows = np.asarray(flows, dtype=np.float32)
    W1 = np.asarray(W1, dtype=np.float32)
    W2 = np.asarray(W2, dtype=np.float32)
    W3 = np.asarray(W3, dtype=np.float32)
    b1 = np.asarray(b1, dtype=np.float32)
    b2 = np.asarray(b2, dtype=np.float32)
    b3 = np.asarray(b3, dtype=np.float32)

    nc = _get_nc()

    deg = flows.sum(axis=1)                          # [B, N] column sums
    dinv = (1.0 / np.sqrt(deg)).astype(np.float32)

    # Ahat = D A D, quantized at 2^16; natural + transposed packings
    Aq8 = np.empty((B, N, N), dtype=FP8)
    M1R = np.empty((B, 1, N), dtype=BF16)
    for g in range(B):
        ah = (flows[g] * (dinv[g][:, None] * 2.0 ** 16)) * dinv[g][None, :]
        Aq8[g] = _q8(ah)
        M1R[g, 0] = (Aq8[g].astype(np.float32).sum(axis=0)
                     * 2.0 ** -11).astype(BF16)
    AHp = _pack(Aq8)
    ATp = _pack(np.ascontiguousarray(Aq8.transpose(0, 2, 1)))

    # W1g = 2^6 D^{-1} W1 per graph, dither-quantized per column
    W1g = (np.sqrt(deg)[:, :, None] * W1[None, :, :] * 2.0 ** 6).astype(np.float32)
    W1q = _dither_q8(W1g.transpose(1, 0, 2).reshape(N, B * D))
    W1q = W1q.reshape(N, B, D).transpose(1, 0, 2)    # [B, N, D] fp8
    W1G = np.ascontiguousarray(
        W1q.reshape(B, KB, 2, P, D).transpose(0, 1, 3, 2, 4))

    W23 = ((W2 @ W3) * 2.0 ** 9).astype(BF16)
    SC = (np.sqrt(deg) * 2.0 ** -16).astype(np.float32)       # [B, N]
    SC = np.ascontiguousarray(SC.reshape(B, NB, P).transpose(0, 2, 1))

    in_maps = []
    for c in range(NCORES):
        sl = slice(c * GPC, (c + 1) * GPC)
        in_maps.append({
            "AH": AHp[sl], "AT": ATp[sl],
            "W1G": W1G[sl], "W23": W23,
            "M1R": M1R[sl], "SC": SC[sl],
            "IOB": np.eye(P, dtype=BF16),
        })

    res = run_bass_kernel_spmd(nc, in_maps, core_ids=list(range(NCORES)),
                               trace=_trace)
    out = np.concatenate([res.results[c]["out"] for c in range(NCORES)], axis=0)
    out = np.ascontiguousarray(out.astype(np.float32).transpose(0, 2, 1))

    if np.any(b1) or np.any(b2) or np.any(b3):
        dv = np.where(deg > 0, 1.0 / np.sqrt(deg), 0.0).astype(np.float32)
        m1 = dv * np.einsum('brc,br->bc', flows, dv)
        m2 = dv * np.einsum('brc,br->bc', flows, dv * m1)
        out += m2[..., None] * (b1 @ W2 @ W3)[None, None, :]
        out += m1[..., None] * (b2 @ W3)[None, None, :]
        out += b3[None, None, :]

    if _trace:
        return out, res
    return out
